# revision 1
# baseline (speedup 1.0000x reference)
"""AFM layer kernel for 8 TRN2 NeuronCores.

Math: the reference's attention softmax is over a size-1 axis, so the
attention weights are exactly 1.0 and the attention MLP (Wa, ba, Wh, bh)
cancels out of the output.  What remains is

    pooled[b, :] = sum_{i<j} e_i * e_j          (elementwise over k=16)
                 = 0.5 * ((sum_f e_f)^2 - sum_f e_f^2)
    out[b]       = sigmoid(pooled @ Wo + bo)

where e_f = emb_tables[f, sparse[b, f], :].  The device kernel is an
embedding gather (indirect DMA, one row per partition per instruction —
the only indirect-DMA shape this toolchain lowers correctly) plus a
small amount of vector math.

The table is augmented host-side to rows [e | e^2] (a data-independent
transform), so a single 128B gather descriptor delivers both the value
and its square — no on-chip squaring, which keeps the scalar engine off
the critical path.

Sharding: data-parallel over batch; each of the 8 cores handles 256 rows
(2 half-tiles of 128 partition rows, batch row = h*128 + p).  Embedding
tables are replicated; Wo/bo ride in the same packed input DMA as the
indices.
"""

import numpy as np

try:
    import concourse  # noqa: F401
except ImportError:  # pragma: no cover
    import sys

    sys.path.insert(0, "/opt/trn_rl_repo")

N_FIELDS = 26
VOCAB = 10000
K = 16
BATCH = 2048
N_CORES = 8
PER_CORE = BATCH // N_CORES  # 256
HALVES = PER_CORE // 128  # 2
N_CHUNK = HALVES * N_FIELDS  # 52 gathered rows per partition
CW = 2 * K  # 32 floats per augmented table row [e | e^2]
PACK_W = N_CHUNK + K + 1  # packed input: idx(52) ++ Wo(16) ++ bo(1)

_NC_CACHE = {}


def _build_nc():
    from concourse import bass, mybir

    f32 = mybir.dt.float32
    i32 = mybir.dt.int32

    nc = bass.Bass()
    pack_d = nc.declare_dram_parameter("pack", [128, PACK_W], f32, isOutput=False)
    emb_d = nc.declare_dram_parameter("emb", [N_FIELDS * VOCAB, CW], f32, isOutput=False)
    # out layout: [p, h] — batch row h*128 + p lives at out[p, h]
    out_d = nc.declare_dram_parameter("out", [128, HALVES], f32, isOutput=True)

    with (
        nc.sbuf_tensor([128, PACK_W], f32) as pack_t,
        nc.sbuf_tensor([128, N_CHUNK * CW], f32) as e_t,
        nc.sbuf_tensor([128, HALVES * K], f32) as s_t,
        nc.sbuf_tensor([128, HALVES * K], f32) as q_t,
        nc.sbuf_tensor([128, HALVES * K], f32) as sw_t,
        nc.sbuf_tensor([128, HALVES * K], f32) as ssw_t,
        nc.sbuf_tensor([128, HALVES * K], f32) as qw_t,
        nc.sbuf_tensor([128, HALVES], f32) as t_acc,
        nc.sbuf_tensor([128, HALVES], f32) as u_acc,
        nc.sbuf_tensor([128, HALVES], f32) as d_t,
        nc.sbuf_tensor([128, HALVES], f32) as y_t,
        nc.sbuf_tensor([128, K], f32) as scr_t,
        nc.sbuf_tensor([128, 64], f32) as dscr_t,
        nc.semaphore("i_sem") as i_sem,
        nc.semaphore("g_sem") as g_sem,
        nc.semaphore("v_sem") as v_sem,
        nc.semaphore("o_sem") as o_sem,
        nc.Block(no_gpsimd_drain=True) as block,
    ):
        idx_v = pack_t[:, 0:N_CHUNK].bitcast(i32)  # int32 bits in f32 carrier
        wo_v = pack_t[:, N_CHUNK : N_CHUNK + K]
        bo_v = pack_t[:, N_CHUNK + K : N_CHUNK + K + 1]

        # e_t free layout per partition: [h, f, (e|sq), k]
        e_all = e_t[:, :].rearrange(
            "p (h f g k) -> p g h k f", h=HALVES, f=N_FIELDS, g=2, k=K
        )
        e_hkf = e_all[:, 0]  # [128, h, k, f] — values
        sq_hkf = e_all[:, 1]  # [128, h, k, f] — squares
        s_v = s_t[:, :].rearrange("p (h k) -> p h k", h=HALVES, k=K)
        q_v = q_t[:, :].rearrange("p (h k) -> p h k", h=HALVES, k=K)
        sw_v = sw_t[:, :].rearrange("p (h k) -> p h k", h=HALVES, k=K)
        ssw_v = ssw_t[:, :].rearrange("p (h k) -> p h k", h=HALVES, k=K)
        qw_v = qw_t[:, :].rearrange("p (h k) -> p h k", h=HALVES, k=K)
        t_v = t_acc[:, :].rearrange("p (h o) -> p h o", h=HALVES, o=1)
        u_v = u_acc[:, :].rearrange("p (h o) -> p h o", h=HALVES, o=1)

        @block.sync
        def _(sp):
            sp.dma_start(out=pack_t[:, :], in_=pack_d[:, :]).then_inc(i_sem, 16)

        @block.gpsimd
        def _(g):
            g.wait_ge(i_sem, 16)
            for j in range(N_CHUNK):
                g.indirect_dma_start(
                    out=e_t[:, j * CW : (j + 1) * CW],
                    out_offset=None,
                    in_=emb_d[:, :],
                    in_offset=bass.IndirectOffsetOnAxis(
                        ap=idx_v[:, j : j + 1], axis=0
                    ),
                ).then_inc(g_sem, 16)

        # NOTE on hazards (empirically established on this toolchain):
        # - an instruction reading an SBUF region written by its IMMEDIATELY
        #   preceding same-engine instruction sees stale data (no HW
        #   interlock; engine_nop does not help — real ops do; >=2 real ops
        #   of spacing is verified safe);
        # - a cross-engine consumer gated only by .then_inc on the producing
        #   instruction can also see stale data, so handoff sem incs ride on
        #   a drain preceded by >=2 unrelated real ops.
        @block.scalar
        def _(s):
            s.wait_ge(v_sem, 1)
            s.activation(
                y_t[:, :],
                d_t[:, :],
                func=mybir.ActivationFunctionType.Sigmoid,
                bias=bo_v,
                scale=0.5,
            )
            # wide spacer ops + drain so y_t's write lands, then ACT itself
            # issues the output store (HWDGE) — its ~600ns issue latency adds
            # further margin before the SBUF read
            s.activation(
                scr_t[:, 0:K],
                wo_v,
                func=mybir.ActivationFunctionType.Sigmoid,
                bias=bo_v,
                scale=1.0,
            )
            s.activation(
                scr_t[:, 0:K],
                wo_v,
                func=mybir.ActivationFunctionType.Sigmoid,
                bias=bo_v,
                scale=1.0,
            )
            s.drain()
            s.dma_start(out=out_d[:, :], in_=y_t[:, :]).then_inc(o_sem, 16)
            s.wait_ge(o_sem, 16)

        @block.vector
        def _(v):
            v.wait_ge(i_sem, 16)  # wo available
            # h0 chain while h1 is still gathering
            v.wait_ge(g_sem, 16 * N_FIELDS)
            v.reduce_sum(s_v[:, 0], e_hkf[:, 0], axis=mybir.AxisListType.X)
            v.reduce_sum(q_v[:, 0], sq_hkf[:, 0], axis=mybir.AxisListType.X)
            v.tensor_mul(out=sw_v[:, 0], in0=s_v[:, 0], in1=wo_v)
            v.tensor_mul(out=qw_v[:, 0], in0=q_v[:, 0], in1=wo_v)
            # wide dummy (~240ns) so ssw/u read sw/qw outside the stale window
            v.tensor_mul(out=dscr_t[:, :], in0=e_t[:, 0:64], in1=e_t[:, 0:64])
            v.tensor_mul(out=ssw_v[:, 0], in0=s_v[:, 0], in1=sw_v[:, 0])
            v.reduce_sum(u_v[:, 0], qw_v[:, 0], axis=mybir.AxisListType.X)
            v.tensor_mul(out=dscr_t[:, :], in0=e_t[:, 64:128], in1=e_t[:, 64:128])
            v.reduce_sum(t_v[:, 0], ssw_v[:, 0], axis=mybir.AxisListType.X)
            # h1 tail
            v.wait_ge(g_sem, 16 * N_CHUNK)
            v.reduce_sum(s_v[:, 1], e_hkf[:, 1], axis=mybir.AxisListType.X)
            v.reduce_sum(q_v[:, 1], sq_hkf[:, 1], axis=mybir.AxisListType.X)
            v.tensor_mul(out=sw_v[:, 1], in0=s_v[:, 1], in1=wo_v)
            v.tensor_mul(out=qw_v[:, 1], in0=q_v[:, 1], in1=wo_v)
            # wide dummy (~240ns) so ssw/u read sw/qw outside the stale window
            v.tensor_mul(out=dscr_t[:, :], in0=e_t[:, 0:64], in1=e_t[:, 0:64])
            v.tensor_mul(out=ssw_v[:, 1], in0=s_v[:, 1], in1=sw_v[:, 1])
            v.reduce_sum(u_v[:, 1], qw_v[:, 1], axis=mybir.AxisListType.X)
            v.tensor_mul(out=dscr_t[:, :], in0=e_t[:, 64:128], in1=e_t[:, 64:128])
            v.reduce_sum(t_v[:, 1], ssw_v[:, 1], axis=mybir.AxisListType.X)
            # wide spacers (full 32-elem tiles, ~3 deep) so t_acc's writes
            # land before the sub reads them — the stale-read window is
            # time-based (~200ns), so short 16-elem ops are not enough
            v.tensor_mul(out=qw_t[:, :], in0=q_t[:, :], in1=q_t[:, :])
            v.tensor_mul(out=ssw_t[:, :], in0=s_t[:, :], in1=s_t[:, :])
            v.tensor_mul(out=sw_t[:, :], in0=q_t[:, :], in1=s_t[:, :])
            v.tensor_sub(out=d_t[:, :], in0=t_acc[:, :], in1=u_acc[:, :])
            # wide spacers so d_t's write lands before the drain's sem inc
            v.tensor_mul(out=qw_t[:, :], in0=q_t[:, :], in1=s_t[:, :])
            v.tensor_mul(out=ssw_t[:, :], in0=s_t[:, :], in1=q_t[:, :])
            v.tensor_mul(out=sw_t[:, :], in0=s_t[:, :], in1=s_t[:, :])
            v.drain().then_inc(v_sem, 1)

    return nc


def _get_nc():
    if "nc" not in _NC_CACHE:
        _NC_CACHE["nc"] = _build_nc()
    return _NC_CACHE["nc"]


def _prep_in_maps(sparse, emb_tables, Wo, bo):
    sparse = np.asarray(sparse)
    emb_flat = np.asarray(emb_tables, dtype=np.float32).reshape(N_FIELDS * VOCAB, K)
    emb_aug = np.empty((N_FIELDS * VOCAB, CW), dtype=np.float32)
    emb_aug[:, 0:K] = emb_flat
    emb_aug[:, K:CW] = emb_flat * emb_flat

    # flat row index into the stacked [26*10000, 32] table
    flat_idx = (
        sparse.astype(np.int32) + (np.arange(N_FIELDS, dtype=np.int32) * VOCAB)[None, :]
    )  # [2048, 26]

    wo_row = np.asarray(Wo, dtype=np.float32).reshape(K)
    bo_val = np.float32(np.asarray(bo).reshape(-1)[0])

    in_maps = []
    for c in range(N_CORES):
        rows = flat_idx[c * PER_CORE : (c + 1) * PER_CORE]  # [256, 26]
        # [h, p, f] -> [p, (h f)]
        idx_c = np.ascontiguousarray(
            rows.reshape(HALVES, 128, N_FIELDS).transpose(1, 0, 2).reshape(128, N_CHUNK)
        )
        pack = np.empty((128, PACK_W), dtype=np.float32)
        pack[:, 0:N_CHUNK] = idx_c.view(np.float32)
        pack[:, N_CHUNK : N_CHUNK + K] = wo_row[None, :]
        pack[:, N_CHUNK + K] = bo_val
        in_maps.append({"pack": pack, "emb": emb_aug})
    return in_maps


def _run(in_maps, trace=False, **kwargs):
    from concourse.bass_utils import run_bass_kernel_spmd

    nc = _get_nc()
    return run_bass_kernel_spmd(
        nc, in_maps, core_ids=list(range(N_CORES)), trace=trace, **kwargs
    )


def _collect_out(res):
    # res out[c] is [128, HALVES]; batch row c*256 + h*128 + p = out[c][p, h]
    return np.concatenate(
        [res.results[c]["out"].T.reshape(PER_CORE, 1) for c in range(N_CORES)], axis=0
    ).astype(np.float32)


def kernel(dense, sparse, emb_tables, Wa, ba, Wh, bh, Wo, bo):
    in_maps = _prep_in_maps(sparse, emb_tables, Wo, bo)
    res = _run(in_maps)
    return _collect_out(res)



# revision 2
# speedup vs baseline: 1.0188x; 1.0188x over previous
"""AFM layer kernel for 8 TRN2 NeuronCores — dma_gather, overlapped reduce.

Math: attention softmax over size-1 axis == 1, so
    pooled[b, :] = 0.5 * ((sum_f e_f)^2 - sum_f e_f^2)
    out[b]       = sigmoid(pooled @ Wo + bo)

Gather: 12 dma_gather calls on 4 SWDGE queues, 3 calls per queue
(768 + 768 + 128 idxs = 1664 per queue).  Q7 descriptor generation is
~8.6ns/idx per queue (+~0.4us/call), queues run concurrently, so the
balanced per-queue chain sets the gather makespan (~15.5us).
768-idx calls use 3-field table windows (30000 rows < 2^15, int16);
the final round gathers fields 24/25 as four 128-idx half calls.
Row j of a call lands at partition j%128, col j//128 -> global free
col = 2f + h.

The vector engine reduces in 3 chunks (fields 0-11, 12-23, 24-25) as
each gather round's data lands, hiding most reduce time under gathers.
Augmented table rows are [e(16) | e^2(16) | pad(32)] f32 = 256B.
"""

import numpy as np

try:
    import concourse  # noqa: F401
except ImportError:  # pragma: no cover
    import sys

    sys.path.insert(0, "/opt/trn_rl_repo")

N_FIELDS = 26
VOCAB = 10000
K = 16
BATCH = 2048
N_CORES = 8
PER_CORE = BATCH // N_CORES  # 256
HALVES = PER_CORE // 128  # 2
N_CHUNK = HALVES * N_FIELDS  # 52 gathered rows per partition
CW = 64  # padded table row: 64 f32 = 256B

# rounds of 4 calls (one per queue), descending sizes so late rounds'
# transfers drain quickly: r0 4x768 (f0-11), r1 4x512 (f12-19),
# r2 4x256 (f20-23), r3 4x128 half-calls (f24-25).
# Per queue: 768+512+256+128 = 1664 idxs.
# CALLS entries: (f_lo, n_fields, h_lo, n_idx, queue)
CALLS = (
    [(3 * w, 3, 0, 768, w) for w in range(4)]
    + [(12 + 2 * w, 2, 0, 512, w) for w in range(4)]
    + [(20 + w, 1, 0, 256, w) for w in range(4)]
    + [(24, 1, 0, 128, 0), (24, 1, 1, 128, 1), (25, 1, 0, 128, 2), (25, 1, 1, 128, 3)]
)
# reduce chunks: (f_lo, n_fields, wait_count_per_queue)
CHUNKS = [(0, 12, 1), (12, 8, 2), (20, 4, 3), (24, 2, 4)]

IDX_COLS_F32 = sum(n // 16 for (_, _, _, n, _) in CALLS) // 2  # 208
PACK_W = IDX_COLS_F32 + K + 1  # idx ++ Wo(16) ++ bo(1)

_NC_CACHE = {}


def _build_nc():
    from concourse import bass, mybir
    from concourse.library_config import mlp

    f32 = mybir.dt.float32
    i16 = mybir.dt.int16

    nc = bass.Bass(num_swdge_queues=4)
    pack_d = nc.declare_dram_parameter("pack", [128, PACK_W], f32, isOutput=False)
    emb_d = nc.declare_dram_parameter("emb", [N_FIELDS * VOCAB, CW], f32, isOutput=False)
    out_d = nc.declare_dram_parameter("out", [128, HALVES], f32, isOutput=True)

    NCH = len(CHUNKS)

    from contextlib import ExitStack

    with ExitStack() as stack:
        sb = lambda name, shape: stack.enter_context(  # noqa: E731
            nc.sbuf_tensor(name, shape, f32)
        )
        pack_t = sb("pack_t", [128, PACK_W])
        e_t = sb("e_t", [128, N_CHUNK * CW])
        cs_t = sb("cs_t", [128, NCH * HALVES * K])
        cq_t = sb("cq_t", [128, NCH * HALVES * K])
        s_t = sb("s_t", [128, HALVES * K])
        q_t = sb("q_t", [128, HALVES * K])
        sw_t = sb("sw_t", [128, HALVES * K])
        ssw_t = sb("ssw_t", [128, HALVES * K])
        qw_t = sb("qw_t", [128, HALVES * K])
        t_acc = sb("t_acc", [128, HALVES])
        u_acc = sb("u_acc", [128, HALVES])
        d_t = sb("d_t", [128, HALVES])
        y_t = sb("y_t", [128, HALVES])
        scr_t = sb("scr_t", [128, K])
        dscr_t = sb("dscr_t", [128, 64])
        i_sem = stack.enter_context(nc.semaphore("i_sem"))
        g_sem0 = stack.enter_context(nc.semaphore("g_sem0"))
        g_sem1 = stack.enter_context(nc.semaphore("g_sem1"))
        g_sem2 = stack.enter_context(nc.semaphore("g_sem2"))
        g_sem3 = stack.enter_context(nc.semaphore("g_sem3"))
        v_sem = stack.enter_context(nc.semaphore("v_sem"))
        o_sem = stack.enter_context(nc.semaphore("o_sem"))
        block = stack.enter_context(nc.Block(no_gpsimd_drain=True))
        wo_v = pack_t[:, IDX_COLS_F32 : IDX_COLS_F32 + K]
        bo_v = pack_t[:, IDX_COLS_F32 + K : IDX_COLS_F32 + K + 1]

        # e_t free layout per partition: [c=2f+h, (e|sq|pad), k]
        e_all = e_t[:, :].rearrange(
            "p (f h g k) -> p g h k f", f=N_FIELDS, h=HALVES, g=4, k=K
        )
        e_hkf = e_all[:, 0]  # [128, h, k, f] — values
        sq_hkf = e_all[:, 1]  # [128, h, k, f] — squares
        cs_v = cs_t[:, :].rearrange("p (c h k) -> p c h k", c=NCH, h=HALVES, k=K)
        cq_v = cq_t[:, :].rearrange("p (c h k) -> p c h k", c=NCH, h=HALVES, k=K)
        s_v = s_t[:, :].rearrange("p (h k) -> p h k", h=HALVES, k=K)
        q_v = q_t[:, :].rearrange("p (h k) -> p h k", h=HALVES, k=K)
        sw_v = sw_t[:, :].rearrange("p (h k) -> p h k", h=HALVES, k=K)
        ssw_v = ssw_t[:, :].rearrange("p (h k) -> p h k", h=HALVES, k=K)
        qw_v = qw_t[:, :].rearrange("p (h k) -> p h k", h=HALVES, k=K)
        t_v = t_acc[:, :].rearrange("p (h o) -> p h o", h=HALVES, o=1)
        u_v = u_acc[:, :].rearrange("p (h o) -> p h o", h=HALVES, o=1)

        g_sems = [g_sem0, g_sem1, g_sem2, g_sem3]

        @block.sync
        def _(sp):
            sp.dma_start(out=pack_t[:, :], in_=pack_d[:, :]).then_inc(i_sem, 16)

        @block.gpsimd
        def _(g):
            g.load_library(mlp)
            g.wait_ge(i_sem, 16)
            icol16 = 0
            for (f_lo, nf, h_lo, nidx, q) in CALLS:
                ncols = nidx // 128
                ncols16 = nidx // 16
                icol_f32 = icol16 // 2
                idx_ap = pack_t[:, icol_f32 : icol_f32 + ncols16 // 2].bitcast(i16)
                col = 2 * f_lo + h_lo
                out_ap = e_t[:, col * CW : (col + ncols) * CW].rearrange(
                    "p (c e) -> p c e", c=ncols, e=CW
                )
                in_ap = emb_d[f_lo * VOCAB : (f_lo + nf) * VOCAB, :]
                g.dma_gather(
                    out_ap,
                    in_ap,
                    idx_ap,
                    nidx,
                    nidx,
                    CW,
                    queue_num=q,
                    single_packet=False,
                ).then_inc(g_sems[q], 16)
                icol16 += ncols16

        @block.scalar
        def _(s):
            # dummy activation: hoist ACT_TABLE_LOAD off the critical path
            s.wait_ge(i_sem, 16)
            s.activation(
                scr_t[:, 0:K],
                wo_v,
                func=mybir.ActivationFunctionType.Sigmoid,
                bias=bo_v,
                scale=1.0,
            )
            s.wait_ge(v_sem, 1)
            s.activation(
                y_t[:, :],
                d_t[:, :],
                func=mybir.ActivationFunctionType.Sigmoid,
                bias=bo_v,
                scale=0.5,
            )
            # spacer ops + drain so y_t's write lands before the output DMA
            s.activation(
                scr_t[:, 0:K],
                wo_v,
                func=mybir.ActivationFunctionType.Sigmoid,
                bias=bo_v,
                scale=1.0,
            )
            s.activation(
                scr_t[:, 0:K],
                wo_v,
                func=mybir.ActivationFunctionType.Sigmoid,
                bias=bo_v,
                scale=1.0,
            )
            s.activation(
                scr_t[:, 0:K],
                wo_v,
                func=mybir.ActivationFunctionType.Sigmoid,
                bias=bo_v,
                scale=1.0,
            )
            s.dma_start(out=out_d[:, :], in_=y_t[:, :]).then_inc(o_sem, 16)
            s.wait_ge(o_sem, 16)

        @block.vector
        def _(v):
            v.wait_ge(i_sem, 16)  # wo available
            # per-chunk partial reduces as each gather round lands
            for ci, (f_lo, nf, wcount) in enumerate(CHUNKS):
                for q in range(4):
                    v.wait_ge(g_sems[q], 16 * wcount)
                for h in range(HALVES):
                    v.reduce_sum(
                        cs_v[:, ci, h],
                        e_hkf[:, h, :, f_lo : f_lo + nf],
                        axis=mybir.AxisListType.X,
                    )
                    v.reduce_sum(
                        cq_v[:, ci, h],
                        sq_hkf[:, h, :, f_lo : f_lo + nf],
                        axis=mybir.AxisListType.X,
                    )
            # combine chunks (writes spaced from their readers)
            HK = HALVES * K
            v.tensor_add(out=sw_t[:, :], in0=cs_t[:, 0:HK], in1=cs_t[:, HK : 2 * HK])
            v.tensor_add(out=qw_t[:, :], in0=cq_t[:, 0:HK], in1=cq_t[:, HK : 2 * HK])
            v.tensor_add(out=ssw_t[:, :], in0=cs_t[:, 2 * HK : 3 * HK], in1=cs_t[:, 3 * HK : 4 * HK])
            v.tensor_add(out=dscr_t[:, 0:HK], in0=cq_t[:, 2 * HK : 3 * HK], in1=cq_t[:, 3 * HK : 4 * HK])
            v.tensor_mul(out=dscr_t[:, 32:64], in0=e_t[:, 0:32], in1=e_t[:, 0:32])
            v.tensor_add(out=s_t[:, :], in0=sw_t[:, :], in1=ssw_t[:, :])
            v.tensor_add(out=q_t[:, :], in0=qw_t[:, :], in1=dscr_t[:, 0:HK])
            v.tensor_mul(out=dscr_t[:, 32:64], in0=e_t[:, 0:32], in1=e_t[:, 0:32])
            v.tensor_mul(out=dscr_t[:, 32:64], in0=e_t[:, 32:64], in1=e_t[:, 32:64])
            for h in range(HALVES):
                v.tensor_mul(out=sw_v[:, h], in0=s_v[:, h], in1=wo_v)
                v.tensor_mul(out=qw_v[:, h], in0=q_v[:, h], in1=wo_v)
                v.tensor_mul(out=dscr_t[:, :], in0=e_t[:, 0:64], in1=e_t[:, 0:64])
                v.tensor_mul(out=ssw_v[:, h], in0=s_v[:, h], in1=sw_v[:, h])
                v.reduce_sum(u_v[:, h], qw_v[:, h], axis=mybir.AxisListType.X)
                v.tensor_mul(out=dscr_t[:, :], in0=e_t[:, 64:128], in1=e_t[:, 64:128])
                v.reduce_sum(t_v[:, h], ssw_v[:, h], axis=mybir.AxisListType.X)
            # wide spacers so t_acc/u_acc writes land before the sub reads
            v.tensor_mul(out=qw_t[:, :], in0=q_t[:, :], in1=q_t[:, :])
            v.tensor_mul(out=ssw_t[:, :], in0=s_t[:, :], in1=s_t[:, :])
            v.tensor_mul(out=sw_t[:, :], in0=q_t[:, :], in1=s_t[:, :])
            v.tensor_sub(out=d_t[:, :], in0=t_acc[:, :], in1=u_acc[:, :])
            # wide spacers so d_t's write lands before the drain's sem inc
            v.tensor_mul(out=qw_t[:, :], in0=q_t[:, :], in1=s_t[:, :])
            v.tensor_mul(out=ssw_t[:, :], in0=s_t[:, :], in1=q_t[:, :])
            v.tensor_mul(out=sw_t[:, :], in0=s_t[:, :], in1=s_t[:, :])
            v.tensor_mul(out=dscr_t[:, :], in0=e_t[:, 0:64], in1=e_t[:, 0:64]).then_inc(
                v_sem, 1
            )

    # Populate .instr bytes for InstISA subclasses (library-reload MPC) —
    # raw Bass skips this Bacc.compile() pass and walrus rejects empty
    # .instr with "ISA wrong length".
    mybir.codegen_inst_isa_subclasses(nc)
    return nc


def _get_nc():
    if "nc" not in _NC_CACHE:
        _NC_CACHE["nc"] = _build_nc()
    return _NC_CACHE["nc"]


def _prep_in_maps(sparse, emb_tables, Wo, bo):
    sparse = np.asarray(sparse)
    emb_flat = np.asarray(emb_tables, dtype=np.float32).reshape(N_FIELDS * VOCAB, K)
    emb_aug = np.zeros((N_FIELDS * VOCAB, CW), dtype=np.float32)
    emb_aug[:, 0:K] = emb_flat
    emb_aug[:, K : 2 * K] = emb_flat * emb_flat

    wo_row = np.asarray(Wo, dtype=np.float32).reshape(K)
    bo_val = np.float32(np.asarray(bo).reshape(-1)[0])

    in_maps = []
    for c in range(N_CORES):
        rows = sparse[c * PER_CORE : (c + 1) * PER_CORE].astype(np.int32)  # [256, 26]
        pack = np.zeros((128, PACK_W), dtype=np.float32)
        icol16 = 0
        idx16_all = np.full((16, 2 * IDX_COLS_F32), -1, dtype=np.int16)
        for (f_lo, nf, h_lo, nidx, _q) in CALLS:
            ncols16 = nidx // 16
            if nf > 1:
                # j = t*256 + b_local; value = t*VOCAB + sparse[b, f_lo+t]
                vals = np.concatenate(
                    [
                        (rows[:, f_lo + t] + t * VOCAB).astype(np.int16)
                        for t in range(nf)
                    ]
                )
            else:
                vals = rows[h_lo * 128 : h_lo * 128 + nidx, f_lo].astype(np.int16)
            blk = vals.reshape(ncols16, 16).T  # [16, ncols16]
            idx16_all[:, icol16 : icol16 + ncols16] = blk
            icol16 += ncols16
        idx16_rep = np.tile(idx16_all, (8, 1))  # replicate across 128 partitions
        pack[:, 0:IDX_COLS_F32] = idx16_rep.view(np.float32)
        pack[:, IDX_COLS_F32 : IDX_COLS_F32 + K] = wo_row[None, :]
        pack[:, IDX_COLS_F32 + K] = bo_val
        in_maps.append({"pack": pack, "emb": emb_aug})
    return in_maps


def _run(in_maps, trace=False, **kwargs):
    from concourse.bass_utils import run_bass_kernel_spmd

    nc = _get_nc()
    return run_bass_kernel_spmd(
        nc, in_maps, core_ids=list(range(N_CORES)), trace=trace, **kwargs
    )


def _collect_out(res):
    return np.concatenate(
        [res.results[c]["out"].T.reshape(PER_CORE, 1) for c in range(N_CORES)], axis=0
    ).astype(np.float32)


def kernel(dense, sparse, emb_tables, Wa, ba, Wh, bh, Wo, bo):
    in_maps = _prep_in_maps(sparse, emb_tables, Wo, bo)
    res = _run(in_maps)
    return _collect_out(res)
